# revision 18
# baseline (speedup 1.0000x reference)
"""GCN (2 dense + 3 sparse layers + log_softmax) on 8 Trainium2 NeuronCores.

v2 strategy:
- Nodes (dst) row-sharded 8 ways.  Each aggregation runs densely on the PE as
  out_T[f, t] = sum_s H'[s, f] * B[s, t], B the count-valued adjacency in fp8
  (streamed as moving operand), H' the diag-scaled features in fp8e4
  (stationary operand) with per-layer power-of-2 prescales folded into the
  host-side normalization vectors.
- ONE SBUF-resident B buffer (141KB/partition as 36 rotating group tiles):
  the dense-orientation matrix loads once and serves BOTH dense layers
  (L1 streams/loads, L2 reuses); the buffer is then overwritten in place by
  the sparse-orientation matrix for L3/L4/L5.  HBM traffic: 92MB -> 37MB.
- L4 (d=128) uses fp8 DoubleRow matmuls (2 k-chunks per instruction);
  d=32/64 layers use tile_position column-group concurrency instead.
- Per-layer feature exchange via two AllGathers (8-block A half, 4-block B
  half) pipelined under the aggregation tail.
"""

import os
import numpy as np
import ml_dtypes

import concourse.bacc as bacc
import concourse.mybir as mybir
import concourse.tile as tile
from concourse.bass_utils import run_bass_kernel_spmd

# ---- problem constants ----
N = 12000
NP = 12288                     # padded nodes (96 * 128)
NCORES = 8
NLOC = NP // NCORES            # 1536 rows per core
KC = NP // 128                 # 96 k-chunks
MC = NLOC // 128               # 12 local row chunks
NT = NLOC // 512               # 3 psum col tiles
NG = KC // 8                   # 12 load groups (8 chunks each) per col tile
F_IN = 512
CLS = 6

D1, D2, D3, D4, D5 = 32, 32, 64, 128, 32   # aggregation widths per layer
SA, SB = 32.0, 512.0           # h prescales (folded into norm vectors)


F8 = mybir.dt.float8e4
F16 = mybir.dt.float16
F32 = mybir.dt.float32
NP_F8 = ml_dtypes.float8_e4m3
NP_F16 = np.float16

_cached = {}


def _build_program():
    nc = bacc.Bacc("TRN2", target_bir_lowering=False, debug=False,
                   num_devices=NCORES)

    bden = nc.dram_tensor("bden", [NT, NG, 128, 8 * 512], F8,
                          kind="ExternalInput")
    bsp = nc.dram_tensor("bsp", [NT, NG, 128, 8 * 512], F8,
                         kind="ExternalInput")
    featT = nc.dram_tensor("featT", [2, 4, 128, 768], F16,
                           kind="ExternalInput")
    w1 = nc.dram_tensor("w1", [4, 128, 32], F16, kind="ExternalInput")
    w12b = nc.dram_tensor("w12b", [33, 64], F16, kind="ExternalInput")
    w13b = nc.dram_tensor("w13b", [65, 128], F16, kind="ExternalInput")
    w14 = nc.dram_tensor("w14", [128, 128], F16, kind="ExternalInput")
    w2 = nc.dram_tensor("w2", [128, CLS], F16, kind="ExternalInput")
    biases_pp = nc.dram_tensor("biases_pp", [128, 3], F32,
                               kind="ExternalInput")
    dispp_pre = nc.dram_tensor("dispp_pre", [128, MC], F32,
                               kind="ExternalInput")
    dinvpp_pre = nc.dram_tensor("dinvpp_pre", [128, MC], F32,
                                kind="ExternalInput")
    disr_post = nc.dram_tensor("disr_post", [128, NLOC], F16,
                               kind="ExternalInput")
    disr_pre = nc.dram_tensor("disr_pre", [128, NLOC], F16,
                              kind="ExternalInput")
    dinvr_post = nc.dram_tensor("dinvr_post", [128, NLOC], F16,
                                kind="ExternalInput")
    dinvr_pre = nc.dram_tensor("dinvr_pre", [128, NLOC], F16,
                               kind="ExternalInput")
    ident16 = nc.dram_tensor("ident16", [128, 128], F16, kind="ExternalInput")
    ident32 = nc.dram_tensor("ident32", [128, 128], F32, kind="ExternalInput")
    out = nc.dram_tensor("out", [NLOC, CLS], F32, kind="ExternalOutput")

    AG = mybir.AluOpType
    AF = mybir.ActivationFunctionType
    DR = mybir.MatmulPerfMode.DoubleRow
    RG = [list(range(NCORES))]

    with tile.TileContext(nc) as tc:
        with (
            tc.tile_pool(name="bres", bufs=NT * NG) as bpool,
            tc.tile_pool(name="const", bufs=1) as cpool,
            tc.tile_pool(name="hfull", bufs=1) as hpool,
            tc.tile_pool(name="hloc", bufs=1) as lpool,
            tc.tile_pool(name="fz", bufs=1) as fzpool,
            tc.tile_pool(name="gwork", bufs=4) as gpool,
            tc.tile_pool(name="small", bufs=1) as spool,
            tc.tile_pool(name="agg", bufs=3, space="PSUM") as aggp,
            tc.tile_pool(name="wmm", bufs=2, space="PSUM") as wmmp,
            tc.tile_pool(name="tp", bufs=1, space="PSUM") as tpp,
            tc.tile_pool(name="dram", bufs=1, space="DRAM") as dpool,
        ):
            # ---------- constants ----------
            w1_sb = cpool.tile([128, 4 * 32], F16, tag="w1")
            nc.scalar.dma_start(w1_sb[:].rearrange("p (c j) -> p c j", c=4),
                                w1.ap().rearrange("c p j -> p c j"))
            w12_sb = cpool.tile([33, 64], F16, tag="w12")
            nc.scalar.dma_start(w12_sb[:], w12b[:, :])
            w13_sb = cpool.tile([65, 128], F16, tag="w13")
            nc.scalar.dma_start(w13_sb[:], w13b[:, :])
            w14_sb = cpool.tile([128, 128], F16, tag="w14")
            nc.scalar.dma_start(w14_sb[:], w14[:, :])
            w2_sb = cpool.tile([128, CLS], F16, tag="w2")
            nc.scalar.dma_start(w2_sb[:], w2[:, :])
            bias_sb = cpool.tile([128, 3], F32, tag="bias")
            nc.scalar.dma_start(bias_sb[:], biases_pp[:, :])
            dispp_sb = cpool.tile([128, MC], F32, tag="dispp")
            nc.scalar.dma_start(dispp_sb[:], dispp_pre[:, :])
            dinvpp_sb = cpool.tile([128, MC], F32, tag="dinvpp")
            nc.scalar.dma_start(dinvpp_sb[:], dinvpp_pre[:, :])
            dispo_sb = cpool.tile([128, NLOC], F16, tag="dispo")
            nc.scalar.dma_start(dispo_sb[:], disr_post[:, :])
            dispr_sb = cpool.tile([128, NLOC], F16, tag="dispr")
            nc.scalar.dma_start(dispr_sb[:], disr_pre[:, :])
            dinvpo_sb = cpool.tile([128, NLOC], F16, tag="dinvpo")
            nc.scalar.dma_start(dinvpo_sb[:], dinvr_post[:, :])
            dinvpr_sb = cpool.tile([128, NLOC], F16, tag="dinvpr")
            nc.scalar.dma_start(dinvpr_sb[:], dinvr_pre[:, :])
            id16_sb = cpool.tile([128, 128], F16, tag="id16")
            nc.scalar.dma_start(id16_sb[:], ident16[:, :])
            id32_sb = cpool.tile([128, 128], F32, tag="id32")
            nc.scalar.dma_start(id32_sb[:], ident32[:, :])

            # ---------- h-feature buffers ----------
            h1 = hpool.tile([128, KC * D1], F8, tag="h1")
            h2 = hpool.tile([128, KC * D2], F8, tag="h2")
            h3 = hpool.tile([128, KC * D3], F8, tag="h3")
            h4 = hpool.tile([128, KC * D4], F8, tag="h4")
            h5 = hpool.tile([128, KC * D5], F8, tag="h5")
            h1loc = lpool.tile([128, MC * D1], F8, tag="h1loc")
            h2loc = lpool.tile([128, MC * D2], F8, tag="h2loc")
            h3loc = lpool.tile([128, MC * D3], F8, tag="h3loc")
            h4loc = lpool.tile([128, MC * D4], F8, tag="h4loc")
            h5loc = lpool.tile([128, MC * D5], F8, tag="h5loc")

            # ---------- resident B: 36 group tiles, 2 generations ----------
            ldq = [nc.sync, nc.gpsimd]

            def load_b(mat, gen):
                tiles = [[None] * NG for _ in range(NT)]
                for t in range(NT):
                    for g in range(NG):
                        bt = bpool.tile([128, 8 * 512], F8, tag="br",
                                        name=f"br_{gen}_{t}_{g}")
                        ldq[(t * NG + g) % 2].dma_start(bt[:], mat[t, g])
                        tiles[t][g] = bt
                return tiles

            bden_t = load_b(bden, 0)

            # ---------- L1 local transform: h1 = fp8(SA*dis*(X0@W1)) --------
            feat_sb = fzpool.tile([128, 4 * 768], F16, tag="feat")
            for half in range(2):
                nc.scalar.dma_start(
                    feat_sb[:].rearrange("p (kc x) -> p kc x", kc=4),
                    featT[half].rearrange("kc p x -> p kc x"))
                for m in range(6 * half, 6 * half + 6):
                    mm = m - 6 * half
                    t1 = wmmp.tile([128, 32], F32, tag="wmm", name=f"t1_{m}")
                    for kc in range(4):
                        nc.tensor.matmul(
                            t1[:, :],
                            feat_sb[:, kc * 768 + mm * 128:
                                    kc * 768 + (mm + 1) * 128],
                            w1_sb[:, kc * 32:(kc + 1) * 32],
                            start=(kc == 0), stop=(kc == 3))
                    nc.vector.tensor_scalar_mul(
                        h1loc[:, m * D1:(m + 1) * D1], t1[:, :],
                        dispp_sb[:, m:m + 1])

            # ---------- exchange helper ----------
            def exchange(hloc, hfull, d, lname):
                w = MC * d
                bin_t = dpool.tile([128, w], F8, tag=f"xi_{lname}")
                bout_t = dpool.tile([NCORES, 128, w], F8, tag=f"xo_{lname}",
                                    addr_space="Shared")
                nc.scalar.dma_start(bin_t[:], hloc[:, 0:w])
                nc.gpsimd.collective_compute(
                    "AllGather", AG.bypass, replica_groups=RG,
                    ins=[bin_t.opt()], outs=[bout_t.opt()])
                for c in range(NCORES):
                    nc.scalar.dma_start(
                        hfull[:, c * MC * d: (c + 1) * MC * d], bout_t[c])

            exchange(h1loc, h1, D1, "h1")

            # ---------- aggregation pass ----------
            def gsum(a, d, name):
                """Sum the 128//d col-group partials of psum tile a -> SBUF."""
                gs = spool.tile([64, 512], F32, tag="gs", name=name)
                nc.scalar.activation(gs[0:d, :], a[0:d, :], AF.Copy)
                for q in range(1, 128 // d):
                    nc.vector.tensor_tensor(gs[0:d, :], gs[0:d, :],
                                            a[q * d:(q + 1) * d, :], op=AG.add)
                return gs

            def agg_pass(lname, hfull, d, btiles, post_fn, exch_fn, dr=False):
                P4 = 128 // d
                aggs = [aggp.tile([128, 512], F32, tag="agg",
                                  name=f"agg_{lname}_{t}") for t in range(NT)]

                def mms(t):
                    if dr:
                        for j in range(KC // 2):
                            k = 2 * j
                            bt = btiles[t][k // 8]
                            kg = k % 8
                            nc.tensor.matmul(
                                aggs[t][:, :],
                                hfull[:, k * d:(k + 2) * d].rearrange(
                                    "p (two f) -> p two f", two=2),
                                bt[:, kg * 512:(kg + 2) * 512].rearrange(
                                    "p (two n) -> p two n", two=2),
                                start=(k == 0), stop=(k == KC - 2),
                                perf_mode=DR)
                    else:
                        for k in range(KC):
                            q = k % P4
                            bt = btiles[t][k // 8]
                            kg = k % 8
                            nc.tensor.matmul(
                                aggs[t][q * d:(q + 1) * d, :],
                                hfull[:, k * d:(k + 1) * d],
                                bt[:, kg * 512:(kg + 1) * 512],
                                start=(k < P4), stop=(k >= KC - P4),
                                tile_position=(0, q * d))

                mms(0)
                mms(1)
                post_fn(0, aggs[0])
                mms(2)
                post_fn(1, aggs[1])
                post_fn(2, aggs[2])
                if exch_fn is not None:
                    exch_fn()

            # ---------- L1: x1 = relu(dis*G1 + b1); h2 = fp8(SA*dis*x1) -----
            def post1(t, a):
                sl = slice(t * 512, (t + 1) * 512)
                gs = gsum(a, D1, f"gs1_{t}")
                nc.vector.tensor_tensor(gs[0:32, :], gs[0:32, :],
                                        dispo_sb[0:32, sl], op=AG.mult)
                x1p = gpool.tile([128, 512], F16, tag="gw", name=f"x1p_{t}")
                nc.scalar.activation(x1p[0:32, :], gs[0:32, :], AF.Relu,
                                     bias=bias_sb[0:32, 0:1])
                nc.vector.tensor_tensor(x1p[0:32, :], x1p[0:32, :],
                                        dispr_sb[0:32, sl], op=AG.mult)
                tp = tpp.tile([128, 128], F16, tag="tp16", name=f"tp1_{t}")
                for mm in range(4):
                    nc.tensor.transpose(tp[:, mm * 32:(mm + 1) * 32],
                                        x1p[0:32, mm * 128:(mm + 1) * 128],
                                        id16_sb[0:32, 0:32])
                nc.vector.tensor_copy(
                    h2loc[:, (4 * t) * D2:(4 * t + 4) * D2], tp[:, :])

            agg_pass("l1", h1, D1, bden_t, post1,
                     lambda: exchange(h2loc, h2, D2, "h2"))

            # ---------- L2: x2 = relu(dis*G2@W12+b12); h3 = fp8(SB*dinv*x2) -
            def post2(t, a):
                sl = slice(t * 512, (t + 1) * 512)
                gs = gsum(a, D2, f"gs2_{t}")
                g2p = gpool.tile([128, 512], F16, tag="gw", name=f"g2p_{t}")
                nc.vector.tensor_tensor(g2p[0:32, :], gs[0:32, :],
                                        dispo_sb[0:32, sl], op=AG.mult)
                nc.vector.memset(g2p[32:33, :], 1.0)
                for mm in range(4):
                    m = 4 * t + mm
                    xp = wmmp.tile([128, 64], F32, tag="wmm", name=f"x2_{m}")
                    nc.tensor.matmul(xp[:, :],
                                     g2p[0:33, mm * 128:(mm + 1) * 128],
                                     w12_sb[:, :], start=True, stop=True)
                    nc.vector.tensor_scalar(
                        h3loc[:, m * D3:(m + 1) * D3], xp[:, :],
                        0.0, dinvpp_sb[:, m:m + 1], op0=AG.max, op1=AG.mult)

            agg_pass("l2", h2, D2, bden_t, post2,
                     lambda: exchange(h3loc, h3, D3, "h3"))

            # ---------- reload resident B with the sparse-orientation matrix
            bsp_t = load_b(bsp, 1)

            # ---------- L3: x3 = relu(dinv*G3@W13+b13); h4 = fp8(SB*dinv*x3)
            def post3(t, a):
                sl = slice(t * 512, (t + 1) * 512)
                gs = gsum(a, D3, f"gs3_{t}")
                g3p = gpool.tile([128, 512], F16, tag="gw", name=f"g3p_{t}")
                nc.vector.tensor_tensor(g3p[0:64, :], gs[0:64, :],
                                        dinvpo_sb[0:64, sl], op=AG.mult)
                nc.vector.memset(g3p[64:65, :], 1.0)
                for mm in range(4):
                    m = 4 * t + mm
                    xp = wmmp.tile([128, 128], F32, tag="wmm", name=f"x3_{m}")
                    nc.tensor.matmul(xp[:, :],
                                     g3p[0:65, mm * 128:(mm + 1) * 128],
                                     w13_sb[:, :], start=True, stop=True)
                    nc.vector.tensor_scalar(
                        h4loc[:, m * D4:(m + 1) * D4], xp[:, :],
                        0.0, dinvpp_sb[:, m:m + 1], op0=AG.max, op1=AG.mult)

            agg_pass("l3", h3, D3, bsp_t, post3,
                     lambda: exchange(h4loc, h4, D4, "h4"))

            # ---------- L4 (DoubleRow): x4 = relu(dinv*G4@W14+b14) ----------
            # ---------- h5 = fp8(SB*dinv*(x4@W2)), built transposed ---------
            def post4(t, a):
                sl = slice(t * 512, (t + 1) * 512)
                g4p = gpool.tile([128, 512], F16, tag="gw", name=f"g4p_{t}")
                nc.vector.tensor_tensor(g4p[:, :], a[:, :],
                                        dinvpo_sb[:, sl], op=AG.mult)
                x4p = wmmp.tile([128, 512], F32, tag="wmm", name=f"x4p_{t}")
                nc.tensor.matmul(x4p[:, :], w14_sb[:, :], g4p[:, :],
                                 start=True, stop=True)
                x4T = gpool.tile([128, 512], F16, tag="gw", name=f"x4T_{t}")
                nc.scalar.activation(x4T[:, :], x4p[:, :], AF.Relu,
                                     bias=bias_sb[:, 1:2])
                t5 = wmmp.tile([CLS, 512], F32, tag="wmm", name=f"t5_{t}")
                nc.tensor.matmul(t5[:, :], w2_sb[:, :], x4T[:, :],
                                 start=True, stop=True)
                h5T = gpool.tile([128, 512], F16, tag="gw", name=f"h5T_{t}")
                nc.vector.memset(h5T[0:32, :], 0.0)
                nc.vector.tensor_tensor(h5T[0:CLS, :], t5[:, :],
                                        dinvpr_sb[0:CLS, sl], op=AG.mult)
                tp = tpp.tile([128, 128], F16, tag="tp16", name=f"tp5_{t}")
                for mm in range(4):
                    nc.tensor.transpose(tp[:, mm * 32:(mm + 1) * 32],
                                        h5T[0:32, mm * 128:(mm + 1) * 128],
                                        id16_sb[0:32, 0:32])
                nc.vector.tensor_copy(
                    h5loc[:, (4 * t) * D5:(4 * t + 4) * D5], tp[:, :])

            agg_pass("l4", h4, D4, bsp_t, post4,
                     lambda: exchange(h5loc, h5, D5, "h5"), dr=True)

            # ---------- L5: z = dinv*G5 + b2, log_softmax -------------------
            zt = fzpool.tile([32, NLOC], F32, tag="zt")
            nc.vector.memset(zt[0:32, :], 0.0)

            def post5(t, a):
                sl = slice(t * 512, (t + 1) * 512)
                gs = gsum(a, D5, f"gs5_{t}")
                nc.vector.tensor_tensor(zt[0:CLS, sl], gs[0:CLS, :],
                                        dinvpo_sb[0:CLS, sl], op=AG.mult)
                nc.vector.tensor_scalar_add(zt[0:CLS, sl], zt[0:CLS, sl],
                                            bias_sb[0:CLS, 2:3])

            agg_pass("l5", h5, D5, bsp_t, post5, None)

            ztp = tpp.tile([128, MC * 32], F32, tag="tp32")
            outsb = spool.tile([128, MC * CLS], F32, tag="outsb")
            for m in range(MC):
                nc.tensor.transpose(
                    ztp[:, m * 32:(m + 1) * 32],
                    zt[:, m * 128:(m + 1) * 128], id32_sb[0:32, 0:32])
            nmt = spool.tile([128, MC], F32, tag="nmt")
            et = spool.tile([128, MC * CLS], F32, tag="et")
            st = spool.tile([128, MC], F32, tag="st")
            lst = spool.tile([128, MC], F32, tag="lst")
            for m in range(MC):
                nc.vector.reduce_max(nmt[:, m:m + 1],
                                     ztp[:, m * 32: m * 32 + CLS],
                                     axis=mybir.AxisListType.X, negate=True)
            for m in range(MC):
                nc.scalar.activation(et[:, m * CLS:(m + 1) * CLS],
                                     ztp[:, m * 32: m * 32 + CLS], AF.Exp,
                                     bias=nmt[:, m:m + 1])
            nc.vector.reduce_sum(
                st[:, :], et[:].rearrange("p (m f) -> p m f", m=MC),
                axis=mybir.AxisListType.X)
            nc.scalar.activation(lst[:, :], st[:, :], AF.Ln)
            for m in range(MC):
                nc.vector.tensor_scalar(
                    outsb[:, m * CLS:(m + 1) * CLS],
                    ztp[:, m * 32: m * 32 + CLS],
                    nmt[:, m:m + 1], lst[:, m:m + 1],
                    op0=AG.add, op1=AG.subtract)
            nc.scalar.dma_start(
                out.ap().rearrange("(m p) f -> p m f", p=128),
                outsb[:].rearrange("p (m f) -> p m f", m=MC))

    nc.compile()
    return nc


# ---------------------------------------------------------------------------
# host-side preprocessing
# ---------------------------------------------------------------------------

def _preprocess(node_feats, edge_index, W1, b1, W12, b12, W13, b13, W14, b14,
                W2, b2):
    src = np.asarray(edge_index[0], dtype=np.int64)
    dst = np.asarray(edge_index[1], dtype=np.int64)

    # dense-path stream: stream[s, t] = count(t->s) offdiag, diag forced to 1
    Bden = np.zeros(NP * NP, dtype=np.uint8)
    np.add.at(Bden, src * NP + dst, 1)
    Bden = Bden.reshape(NP, NP)
    idx = np.arange(N)
    Bden[idx, idx] = 1
    deg_den = Bden[:N].sum(axis=1, dtype=np.int64).astype(np.float64)
    dis = np.zeros(NP, dtype=np.float64)
    dis[:N] = np.maximum(deg_den, 1.0) ** -0.5
    dis[N:] = 1.0

    # sparse-path stream: stream[s, t] = count(s->t) + I
    Bsp = np.zeros(NP * NP, dtype=np.uint8)
    np.add.at(Bsp, dst * NP + src, 1)
    Bsp = Bsp.reshape(NP, NP)
    Bsp[idx, idx] += 1
    deg_sp = Bsp[:N].sum(axis=1, dtype=np.int64).astype(np.float64)
    dinv = np.zeros(NP, dtype=np.float64)
    dinv[:N] = np.where(deg_sp > 0, deg_sp.astype(np.float64) ** -0.5, 0.0)

    x0 = np.zeros((NP, F_IN), dtype=np.float32)
    x0[:N] = np.asarray(node_feats, dtype=np.float32)

    def pack(Bmat, rows):
        S = Bmat[rows].T.reshape(KC, 128, NLOC)            # [k, p, (t n)]
        S = S.reshape(NG, 8, 128, NT, 512).transpose(3, 0, 2, 1, 4)
        return np.ascontiguousarray(
            S.reshape(NT, NG, 128, 8 * 512)).astype(NP_F8)

    def pp(vec, c):
        loc = vec[c * NLOC:(c + 1) * NLOC].astype(np.float32)
        return np.ascontiguousarray(loc.reshape(MC, 128).T)

    def repl(vec, c):
        loc = vec[c * NLOC:(c + 1) * NLOC].astype(NP_F16)
        return np.ascontiguousarray(np.broadcast_to(loc[None, :], (128, NLOC)))

    w12b = np.concatenate([np.asarray(W12, np.float32),
                           np.asarray(b12, np.float32)[None, :]], axis=0)
    w13b = np.concatenate([np.asarray(W13, np.float32),
                           np.asarray(b13, np.float32)[None, :]], axis=0)
    biases_pp = np.zeros((128, 3), dtype=np.float32)
    biases_pp[:32, 0] = np.asarray(b1, np.float32)
    biases_pp[:, 1] = np.asarray(b14, np.float32)
    biases_pp[:CLS, 2] = np.asarray(b2, np.float32)

    in_maps = []
    for c in range(NCORES):
        rows = slice(c * NLOC, (c + 1) * NLOC)
        featT_c = np.ascontiguousarray(x0[rows].T).reshape(4, 128, NLOC)
        featT_c = np.stack([featT_c[:, :, :768], featT_c[:, :, 768:]])
        in_maps.append({
            "bden": pack(Bden, rows),
            "bsp": pack(Bsp, rows),
            "featT": featT_c.astype(NP_F16),
            "w1": np.asarray(W1, np.float32).reshape(4, 128, 32).astype(NP_F16),
            "w12b": w12b.astype(NP_F16),
            "w13b": w13b.astype(NP_F16),
            "w14": np.asarray(W14, np.float32).astype(NP_F16),
            "w2": np.asarray(W2, np.float32).astype(NP_F16),
            "biases_pp": biases_pp,
            "dispp_pre": pp(dis * SA, c),
            "dinvpp_pre": pp(dinv * SB, c),
            "disr_post": repl(dis / SA, c),
            "disr_pre": repl(dis * SA, c),
            "dinvr_post": repl(dinv / SB, c),
            "dinvr_pre": repl(dinv * SB, c),
            "ident16": np.eye(128, dtype=NP_F16),
            "ident32": np.eye(128, dtype=np.float32),
        })
    return in_maps


def kernel(node_feats, edge_index, W1, b1, W12, b12, W13, b13, W14, b14, W2,
           b2):
    in_maps = _preprocess(node_feats, edge_index, W1, b1, W12, b12, W13, b13,
                          W14, b14, W2, b2)
    if "nc" not in _cached:
        _cached["nc"] = _build_program()
    nc = _cached["nc"]
    trace = bool(int(os.environ.get("KERNEL_TRACE", "0")))
    res = run_bass_kernel_spmd(nc, in_maps, core_ids=list(range(NCORES)),
                               trace=trace)
    _cached["last_result"] = res
    outs = [res.results[c]["out"] for c in range(NCORES)]
    return np.concatenate(outs, axis=0)[:N].astype(np.float32)


# revision 19
# speedup vs baseline: 1.1027x; 1.1027x over previous
"""GCN (2 dense + 3 sparse layers + log_softmax) on 8 Trainium2 NeuronCores.

v2 strategy:
- Nodes (dst) row-sharded 8 ways.  Each aggregation runs densely on the PE as
  out_T[f, t] = sum_s H'[s, f] * B[s, t], B the count-valued adjacency in fp8
  (streamed as moving operand), H' the diag-scaled features in fp8e4
  (stationary operand) with per-layer power-of-2 prescales folded into the
  host-side normalization vectors.
- ONE SBUF-resident B buffer (141KB/partition as 36 rotating group tiles):
  the dense-orientation matrix loads once and serves BOTH dense layers
  (L1 streams/loads, L2 reuses); the buffer is then overwritten in place by
  the sparse-orientation matrix for L3/L4/L5.  HBM traffic: 92MB -> 37MB.
- L4 (d=128) uses fp8 DoubleRow matmuls (2 k-chunks per instruction);
  d=32/64 layers use tile_position column-group concurrency instead.
- Per-layer feature exchange via two AllGathers (8-block A half, 4-block B
  half) pipelined under the aggregation tail.
"""

import os
import numpy as np
import ml_dtypes

import concourse.bacc as bacc
import concourse.mybir as mybir
import concourse.tile as tile
from concourse.bass_utils import run_bass_kernel_spmd

# ---- problem constants ----
N = 12000
NP = 12288                     # padded nodes (96 * 128)
NCORES = 8
NLOC = NP // NCORES            # 1536 rows per core
KC = NP // 128                 # 96 k-chunks
MC = NLOC // 128               # 12 local row chunks
NT = NLOC // 512               # 3 psum col tiles
NG = KC // 8                   # 12 load groups (8 chunks each) per col tile
F_IN = 512
CLS = 6

D1, D2, D3, D4, D5 = 32, 32, 64, 128, 32   # aggregation widths per layer
SA, SB = 32.0, 512.0           # h prescales (folded into norm vectors)


F8 = mybir.dt.float8e4
F16 = mybir.dt.float16
F32 = mybir.dt.float32
NP_F8 = ml_dtypes.float8_e4m3
NP_F16 = np.float16

_cached = {}


def _build_program():
    nc = bacc.Bacc("TRN2", target_bir_lowering=False, debug=False,
                   num_devices=NCORES)

    bden = nc.dram_tensor("bden", [NT, NG, 128, 8 * 512], F8,
                          kind="ExternalInput")
    bsp = nc.dram_tensor("bsp", [NT, NG, 128, 8 * 512], F8,
                         kind="ExternalInput")
    featT = nc.dram_tensor("featT", [2, 4, 128, 768], F16,
                           kind="ExternalInput")
    w1 = nc.dram_tensor("w1", [4, 128, 32], F16, kind="ExternalInput")
    w12b = nc.dram_tensor("w12b", [33, 64], F16, kind="ExternalInput")
    w13b = nc.dram_tensor("w13b", [65, 128], F16, kind="ExternalInput")
    w14 = nc.dram_tensor("w14", [128, 128], F16, kind="ExternalInput")
    w2 = nc.dram_tensor("w2", [128, CLS], F16, kind="ExternalInput")
    biases_pp = nc.dram_tensor("biases_pp", [128, 3], F32,
                               kind="ExternalInput")
    dispp_pre = nc.dram_tensor("dispp_pre", [128, MC], F32,
                               kind="ExternalInput")
    dinvpp_pre = nc.dram_tensor("dinvpp_pre", [128, MC], F32,
                                kind="ExternalInput")
    disr_post = nc.dram_tensor("disr_post", [128, NLOC], F16,
                               kind="ExternalInput")
    disr_pre = nc.dram_tensor("disr_pre", [128, NLOC], F16,
                              kind="ExternalInput")
    dinvr_post = nc.dram_tensor("dinvr_post", [128, NLOC], F16,
                                kind="ExternalInput")
    dinvr_pre = nc.dram_tensor("dinvr_pre", [128, NLOC], F16,
                               kind="ExternalInput")
    ident16 = nc.dram_tensor("ident16", [128, 128], F16, kind="ExternalInput")
    ident32 = nc.dram_tensor("ident32", [128, 128], F32, kind="ExternalInput")
    out = nc.dram_tensor("out", [NLOC, CLS], F32, kind="ExternalOutput")

    AG = mybir.AluOpType
    AF = mybir.ActivationFunctionType
    DR = mybir.MatmulPerfMode.DoubleRow
    RG = [list(range(NCORES))]

    with tile.TileContext(nc) as tc:
        with (
            tc.tile_pool(name="bres", bufs=NT * NG) as bpool,
            tc.tile_pool(name="const", bufs=1) as cpool,
            tc.tile_pool(name="hfull", bufs=1) as hpool,
            tc.tile_pool(name="hloc", bufs=1) as lpool,
            tc.tile_pool(name="fz", bufs=1) as fzpool,
            tc.tile_pool(name="gwork", bufs=4) as gpool,
            tc.tile_pool(name="small", bufs=1) as spool,
            tc.tile_pool(name="agg", bufs=3, space="PSUM") as aggp,
            tc.tile_pool(name="wmm", bufs=2, space="PSUM") as wmmp,
            tc.tile_pool(name="tp", bufs=1, space="PSUM") as tpp,
            tc.tile_pool(name="dram", bufs=1, space="DRAM") as dpool,
        ):
            # ---------- critical-path constants first ----------
            w1_sb = cpool.tile([128, 4 * 32], F16, tag="w1")
            nc.scalar.dma_start(w1_sb[:].rearrange("p (c j) -> p c j", c=4),
                                w1.ap().rearrange("c p j -> p c j"))
            dispp_sb = cpool.tile([128, MC], F32, tag="dispp")
            nc.scalar.dma_start(dispp_sb[:], dispp_pre[:, :])

            # ---------- h-feature buffers ----------
            h1 = hpool.tile([128, KC * D1], F8, tag="h1")
            h2 = hpool.tile([128, KC * D2], F8, tag="h2")
            h3 = hpool.tile([128, KC * D3], F8, tag="h3")
            h4 = hpool.tile([128, KC * D4], F8, tag="h4")
            h5 = hpool.tile([128, KC * D5], F8, tag="h5")
            h1loc = lpool.tile([128, MC * D1], F8, tag="h1loc")
            h2loc = lpool.tile([128, MC * D2], F8, tag="h2loc")
            h3loc = lpool.tile([128, MC * D3], F8, tag="h3loc")
            h4loc = lpool.tile([128, MC * D4], F8, tag="h4loc")
            h5loc = lpool.tile([128, MC * D5], F8, tag="h5loc")

            # ---------- L1 local transform: h1 = fp8(SA*dis*(X0@W1)) --------
            feat_sb = fzpool.tile([128, 4 * 768], F16, tag="feat")
            fq = [nc.scalar, nc.sync]
            for half in range(2):
                fq[half].dma_start(
                    feat_sb[:].rearrange("p (kc x) -> p kc x", kc=4),
                    featT[half].rearrange("kc p x -> p kc x"))
                for m in range(6 * half, 6 * half + 6):
                    mm = m - 6 * half
                    t1 = wmmp.tile([128, 32], F32, tag="wmm", name=f"t1_{m}")
                    for kc in range(4):
                        nc.tensor.matmul(
                            t1[:, :],
                            feat_sb[:, kc * 768 + mm * 128:
                                    kc * 768 + (mm + 1) * 128],
                            w1_sb[:, kc * 32:(kc + 1) * 32],
                            start=(kc == 0), stop=(kc == 3))
                    nc.vector.tensor_scalar_mul(
                        h1loc[:, m * D1:(m + 1) * D1], t1[:, :],
                        dispp_sb[:, m:m + 1])

            # ---------- exchange helpers (send / recv split) ----------
            def exchange_send(hloc, d, lname):
                w = MC * d
                bin_t = dpool.tile([128, w], F8, tag=f"xi_{lname}")
                bout_t = dpool.tile([NCORES, 128, w], F8, tag=f"xo_{lname}",
                                    addr_space="Shared")
                nc.scalar.dma_start(bin_t[:], hloc[:, 0:w])
                nc.gpsimd.collective_compute(
                    "AllGather", AG.bypass, replica_groups=RG,
                    ins=[bin_t.opt()], outs=[bout_t.opt()])
                return bout_t

            def exchange_recv(bout_t, hfull, d):
                for c in range(NCORES):
                    nc.scalar.dma_start(
                        hfull[:, c * MC * d: (c + 1) * MC * d], bout_t[c])

            def exchange(hloc, hfull, d, lname):
                exchange_recv(exchange_send(hloc, d, lname), hfull, d)

            bout_h1 = exchange_send(h1loc, D1, "h1")

            # ---------- remaining constants (needed from the L1 posts on) ---
            w12_sb = cpool.tile([33, 64], F16, tag="w12")
            nc.scalar.dma_start(w12_sb[:], w12b[:, :])
            w13_sb = cpool.tile([65, 128], F16, tag="w13")
            nc.scalar.dma_start(w13_sb[:], w13b[:, :])
            w14_sb = cpool.tile([128, 128], F16, tag="w14")
            nc.scalar.dma_start(w14_sb[:], w14[:, :])
            w2_sb = cpool.tile([128, CLS], F16, tag="w2")
            nc.scalar.dma_start(w2_sb[:], w2[:, :])
            bias_sb = cpool.tile([128, 3], F32, tag="bias")
            nc.scalar.dma_start(bias_sb[:], biases_pp[:, :])
            dinvpp_sb = cpool.tile([128, MC], F32, tag="dinvpp")
            nc.scalar.dma_start(dinvpp_sb[:], dinvpp_pre[:, :])
            dispo_sb = cpool.tile([128, NLOC], F16, tag="dispo")
            nc.scalar.dma_start(dispo_sb[:], disr_post[:, :])
            dispr_sb = cpool.tile([128, NLOC], F16, tag="dispr")
            nc.scalar.dma_start(dispr_sb[:], disr_pre[:, :])
            dinvpo_sb = cpool.tile([128, NLOC], F16, tag="dinvpo")
            nc.scalar.dma_start(dinvpo_sb[:], dinvr_post[:, :])
            dinvpr_sb = cpool.tile([128, NLOC], F16, tag="dinvpr")
            nc.scalar.dma_start(dinvpr_sb[:], dinvr_pre[:, :])
            id16_sb = cpool.tile([128, 128], F16, tag="id16")
            nc.scalar.dma_start(id16_sb[:], ident16[:, :])
            id32_sb = cpool.tile([128, 128], F32, tag="id32")
            nc.scalar.dma_start(id32_sb[:], ident32[:, :])

            # ---------- resident B: 36 group tiles, 2 generations ----------
            # gpsimd carries ONLY collectives; B loads go on sync + scalar.
            ldq = [nc.sync, nc.scalar]

            def load_b(mat, gen):
                tiles = [[None] * NG for _ in range(NT)]
                for t in range(NT):
                    for g in range(NG):
                        bt = bpool.tile([128, 8 * 512], F8, tag="br",
                                        name=f"br_{gen}_{t}_{g}")
                        ldq[(t * NG + g) % 2].dma_start(bt[:], mat[t, g])
                        tiles[t][g] = bt
                return tiles

            bden_t = load_b(bden, 0)
            exchange_recv(bout_h1, h1, D1)

            # ---------- aggregation pass ----------
            def gsum(a, d, name):
                """Sum the 128//d col-group partials of psum tile a -> SBUF."""
                gs = spool.tile([64, 512], F32, tag="gs", name=name)
                nc.scalar.activation(gs[0:d, :], a[0:d, :], AF.Copy)
                for q in range(1, 128 // d):
                    nc.vector.tensor_tensor(gs[0:d, :], gs[0:d, :],
                                            a[q * d:(q + 1) * d, :], op=AG.add)
                return gs

            def agg_pass(lname, hfull, d, btiles, post_fn, exch_fn, dr=False):
                P4 = 128 // d
                aggs = [aggp.tile([128, 512], F32, tag="agg",
                                  name=f"agg_{lname}_{t}") for t in range(NT)]

                def mms(t):
                    if dr:
                        for j in range(KC // 2):
                            k = 2 * j
                            bt = btiles[t][k // 8]
                            kg = k % 8
                            nc.tensor.matmul(
                                aggs[t][:, :],
                                hfull[:, k * d:(k + 2) * d].rearrange(
                                    "p (two f) -> p two f", two=2),
                                bt[:, kg * 512:(kg + 2) * 512].rearrange(
                                    "p (two n) -> p two n", two=2),
                                start=(k == 0), stop=(k == KC - 2),
                                perf_mode=DR)
                    else:
                        for k in range(KC):
                            q = k % P4
                            bt = btiles[t][k // 8]
                            kg = k % 8
                            nc.tensor.matmul(
                                aggs[t][q * d:(q + 1) * d, :],
                                hfull[:, k * d:(k + 1) * d],
                                bt[:, kg * 512:(kg + 1) * 512],
                                start=(k < P4), stop=(k >= KC - P4),
                                tile_position=(0, q * d))

                mms(0)
                mms(1)
                post_fn(0, aggs[0])
                mms(2)
                post_fn(1, aggs[1])
                post_fn(2, aggs[2])
                if exch_fn is not None:
                    exch_fn()

            # ---------- L1: x1 = relu(dis*G1 + b1); h2 = fp8(SA*dis*x1) -----
            def post1(t, a):
                sl = slice(t * 512, (t + 1) * 512)
                gs = gsum(a, D1, f"gs1_{t}")
                nc.vector.tensor_tensor(gs[0:32, :], gs[0:32, :],
                                        dispo_sb[0:32, sl], op=AG.mult)
                x1p = gpool.tile([128, 512], F16, tag="gw", name=f"x1p_{t}")
                nc.scalar.activation(x1p[0:32, :], gs[0:32, :], AF.Relu,
                                     bias=bias_sb[0:32, 0:1])
                nc.vector.tensor_tensor(x1p[0:32, :], x1p[0:32, :],
                                        dispr_sb[0:32, sl], op=AG.mult)
                tp = tpp.tile([128, 128], F16, tag="tp16", name=f"tp1_{t}")
                for mm in range(4):
                    nc.tensor.transpose(tp[:, mm * 32:(mm + 1) * 32],
                                        x1p[0:32, mm * 128:(mm + 1) * 128],
                                        id16_sb[0:32, 0:32])
                nc.vector.tensor_copy(
                    h2loc[:, (4 * t) * D2:(4 * t + 4) * D2], tp[:, :])

            agg_pass("l1", h1, D1, bden_t, post1,
                     lambda: exchange(h2loc, h2, D2, "h2"))

            # ---------- L2: x2 = relu(dis*G2@W12+b12); h3 = fp8(SB*dinv*x2) -
            def post2(t, a):
                sl = slice(t * 512, (t + 1) * 512)
                gs = gsum(a, D2, f"gs2_{t}")
                g2p = gpool.tile([128, 512], F16, tag="gw", name=f"g2p_{t}")
                nc.vector.tensor_tensor(g2p[0:32, :], gs[0:32, :],
                                        dispo_sb[0:32, sl], op=AG.mult)
                nc.vector.memset(g2p[32:33, :], 1.0)
                for mm in range(4):
                    m = 4 * t + mm
                    xp = wmmp.tile([128, 64], F32, tag="wmm", name=f"x2_{m}")
                    nc.tensor.matmul(xp[:, :],
                                     g2p[0:33, mm * 128:(mm + 1) * 128],
                                     w12_sb[:, :], start=True, stop=True)
                    nc.vector.tensor_scalar(
                        h3loc[:, m * D3:(m + 1) * D3], xp[:, :],
                        0.0, dinvpp_sb[:, m:m + 1], op0=AG.max, op1=AG.mult)

            agg_pass("l2", h2, D2, bden_t, post2,
                     lambda: exchange(h3loc, h3, D3, "h3"))

            # ---------- reload resident B with the sparse-orientation matrix
            bsp_t = load_b(bsp, 1)

            # ---------- L3: x3 = relu(dinv*G3@W13+b13); h4 = fp8(SB*dinv*x3)
            def post3(t, a):
                sl = slice(t * 512, (t + 1) * 512)
                gs = gsum(a, D3, f"gs3_{t}")
                g3p = gpool.tile([128, 512], F16, tag="gw", name=f"g3p_{t}")
                nc.vector.tensor_tensor(g3p[0:64, :], gs[0:64, :],
                                        dinvpo_sb[0:64, sl], op=AG.mult)
                nc.vector.memset(g3p[64:65, :], 1.0)
                for mm in range(4):
                    m = 4 * t + mm
                    xp = wmmp.tile([128, 128], F32, tag="wmm", name=f"x3_{m}")
                    nc.tensor.matmul(xp[:, :],
                                     g3p[0:65, mm * 128:(mm + 1) * 128],
                                     w13_sb[:, :], start=True, stop=True)
                    nc.vector.tensor_scalar(
                        h4loc[:, m * D4:(m + 1) * D4], xp[:, :],
                        0.0, dinvpp_sb[:, m:m + 1], op0=AG.max, op1=AG.mult)

            agg_pass("l3", h3, D3, bsp_t, post3,
                     lambda: exchange(h4loc, h4, D4, "h4"))

            # ---------- L4 (DoubleRow): x4 = relu(dinv*G4@W14+b14) ----------
            # ---------- h5 = fp8(SB*dinv*(x4@W2)), built transposed ---------
            def post4(t, a):
                sl = slice(t * 512, (t + 1) * 512)
                g4p = gpool.tile([128, 512], F16, tag="gw", name=f"g4p_{t}")
                nc.vector.tensor_tensor(g4p[:, :], a[:, :],
                                        dinvpo_sb[:, sl], op=AG.mult)
                x4p = wmmp.tile([128, 512], F32, tag="wmm", name=f"x4p_{t}")
                nc.tensor.matmul(x4p[:, :], w14_sb[:, :], g4p[:, :],
                                 start=True, stop=True)
                x4T = gpool.tile([128, 512], F16, tag="gw", name=f"x4T_{t}")
                nc.scalar.activation(x4T[:, :], x4p[:, :], AF.Relu,
                                     bias=bias_sb[:, 1:2])
                t5 = wmmp.tile([CLS, 512], F32, tag="wmm", name=f"t5_{t}")
                nc.tensor.matmul(t5[:, :], w2_sb[:, :], x4T[:, :],
                                 start=True, stop=True)
                h5T = gpool.tile([128, 512], F16, tag="gw", name=f"h5T_{t}")
                nc.vector.memset(h5T[0:32, :], 0.0)
                nc.vector.tensor_tensor(h5T[0:CLS, :], t5[:, :],
                                        dinvpr_sb[0:CLS, sl], op=AG.mult)
                tp = tpp.tile([128, 128], F16, tag="tp16", name=f"tp5_{t}")
                for mm in range(4):
                    nc.tensor.transpose(tp[:, mm * 32:(mm + 1) * 32],
                                        h5T[0:32, mm * 128:(mm + 1) * 128],
                                        id16_sb[0:32, 0:32])
                nc.vector.tensor_copy(
                    h5loc[:, (4 * t) * D5:(4 * t + 4) * D5], tp[:, :])

            agg_pass("l4", h4, D4, bsp_t, post4,
                     lambda: exchange(h5loc, h5, D5, "h5"), dr=True)

            # ---------- L5: z = dinv*G5 + b2, log_softmax -------------------
            zt = fzpool.tile([32, NLOC], F32, tag="zt")
            nc.vector.memset(zt[0:32, :], 0.0)

            def post5(t, a):
                sl = slice(t * 512, (t + 1) * 512)
                gs = gsum(a, D5, f"gs5_{t}")
                nc.vector.tensor_tensor(zt[0:CLS, sl], gs[0:CLS, :],
                                        dinvpo_sb[0:CLS, sl], op=AG.mult)
                nc.vector.tensor_scalar_add(zt[0:CLS, sl], zt[0:CLS, sl],
                                            bias_sb[0:CLS, 2:3])

            agg_pass("l5", h5, D5, bsp_t, post5, None)

            ztp = tpp.tile([128, MC * 32], F32, tag="tp32")
            outsb = spool.tile([128, MC * CLS], F32, tag="outsb")
            for m in range(MC):
                nc.tensor.transpose(
                    ztp[:, m * 32:(m + 1) * 32],
                    zt[:, m * 128:(m + 1) * 128], id32_sb[0:32, 0:32])
            nmt = spool.tile([128, MC], F32, tag="nmt")
            et = spool.tile([128, MC * CLS], F32, tag="et")
            st = spool.tile([128, MC], F32, tag="st")
            lst = spool.tile([128, MC], F32, tag="lst")
            for m in range(MC):
                nc.vector.reduce_max(nmt[:, m:m + 1],
                                     ztp[:, m * 32: m * 32 + CLS],
                                     axis=mybir.AxisListType.X, negate=True)
            for m in range(MC):
                nc.scalar.activation(et[:, m * CLS:(m + 1) * CLS],
                                     ztp[:, m * 32: m * 32 + CLS], AF.Exp,
                                     bias=nmt[:, m:m + 1])
            nc.vector.reduce_sum(
                st[:, :], et[:].rearrange("p (m f) -> p m f", m=MC),
                axis=mybir.AxisListType.X)
            nc.scalar.activation(lst[:, :], st[:, :], AF.Ln)
            for m in range(MC):
                nc.vector.tensor_scalar(
                    outsb[:, m * CLS:(m + 1) * CLS],
                    ztp[:, m * 32: m * 32 + CLS],
                    nmt[:, m:m + 1], lst[:, m:m + 1],
                    op0=AG.add, op1=AG.subtract)
            nc.scalar.dma_start(
                out.ap().rearrange("(m p) f -> p m f", p=128),
                outsb[:].rearrange("p (m f) -> p m f", m=MC))

    nc.compile()
    return nc


# ---------------------------------------------------------------------------
# host-side preprocessing
# ---------------------------------------------------------------------------

def _preprocess(node_feats, edge_index, W1, b1, W12, b12, W13, b13, W14, b14,
                W2, b2):
    src = np.asarray(edge_index[0], dtype=np.int64)
    dst = np.asarray(edge_index[1], dtype=np.int64)

    # dense-path stream: stream[s, t] = count(t->s) offdiag, diag forced to 1
    Bden = np.zeros(NP * NP, dtype=np.uint8)
    np.add.at(Bden, src * NP + dst, 1)
    Bden = Bden.reshape(NP, NP)
    idx = np.arange(N)
    Bden[idx, idx] = 1
    deg_den = Bden[:N].sum(axis=1, dtype=np.int64).astype(np.float64)
    dis = np.zeros(NP, dtype=np.float64)
    dis[:N] = np.maximum(deg_den, 1.0) ** -0.5
    dis[N:] = 1.0

    # sparse-path stream: stream[s, t] = count(s->t) + I
    Bsp = np.zeros(NP * NP, dtype=np.uint8)
    np.add.at(Bsp, dst * NP + src, 1)
    Bsp = Bsp.reshape(NP, NP)
    Bsp[idx, idx] += 1
    deg_sp = Bsp[:N].sum(axis=1, dtype=np.int64).astype(np.float64)
    dinv = np.zeros(NP, dtype=np.float64)
    dinv[:N] = np.where(deg_sp > 0, deg_sp.astype(np.float64) ** -0.5, 0.0)

    x0 = np.zeros((NP, F_IN), dtype=np.float32)
    x0[:N] = np.asarray(node_feats, dtype=np.float32)

    def pack(Bmat, rows):
        S = Bmat[rows].T.reshape(KC, 128, NLOC)            # [k, p, (t n)]
        S = S.reshape(NG, 8, 128, NT, 512).transpose(3, 0, 2, 1, 4)
        return np.ascontiguousarray(
            S.reshape(NT, NG, 128, 8 * 512)).astype(NP_F8)

    def pp(vec, c):
        loc = vec[c * NLOC:(c + 1) * NLOC].astype(np.float32)
        return np.ascontiguousarray(loc.reshape(MC, 128).T)

    def repl(vec, c):
        loc = vec[c * NLOC:(c + 1) * NLOC].astype(NP_F16)
        return np.ascontiguousarray(np.broadcast_to(loc[None, :], (128, NLOC)))

    w12b = np.concatenate([np.asarray(W12, np.float32),
                           np.asarray(b12, np.float32)[None, :]], axis=0)
    w13b = np.concatenate([np.asarray(W13, np.float32),
                           np.asarray(b13, np.float32)[None, :]], axis=0)
    biases_pp = np.zeros((128, 3), dtype=np.float32)
    biases_pp[:32, 0] = np.asarray(b1, np.float32)
    biases_pp[:, 1] = np.asarray(b14, np.float32)
    biases_pp[:CLS, 2] = np.asarray(b2, np.float32)

    in_maps = []
    for c in range(NCORES):
        rows = slice(c * NLOC, (c + 1) * NLOC)
        featT_c = np.ascontiguousarray(x0[rows].T).reshape(4, 128, NLOC)
        featT_c = np.stack([featT_c[:, :, :768], featT_c[:, :, 768:]])
        in_maps.append({
            "bden": pack(Bden, rows),
            "bsp": pack(Bsp, rows),
            "featT": featT_c.astype(NP_F16),
            "w1": np.asarray(W1, np.float32).reshape(4, 128, 32).astype(NP_F16),
            "w12b": w12b.astype(NP_F16),
            "w13b": w13b.astype(NP_F16),
            "w14": np.asarray(W14, np.float32).astype(NP_F16),
            "w2": np.asarray(W2, np.float32).astype(NP_F16),
            "biases_pp": biases_pp,
            "dispp_pre": pp(dis * SA, c),
            "dinvpp_pre": pp(dinv * SB, c),
            "disr_post": repl(dis / SA, c),
            "disr_pre": repl(dis * SA, c),
            "dinvr_post": repl(dinv / SB, c),
            "dinvr_pre": repl(dinv * SB, c),
            "ident16": np.eye(128, dtype=NP_F16),
            "ident32": np.eye(128, dtype=np.float32),
        })
    return in_maps


def kernel(node_feats, edge_index, W1, b1, W12, b12, W13, b13, W14, b14, W2,
           b2):
    in_maps = _preprocess(node_feats, edge_index, W1, b1, W12, b12, W13, b13,
                          W14, b14, W2, b2)
    if "nc" not in _cached:
        _cached["nc"] = _build_program()
    nc = _cached["nc"]
    trace = bool(int(os.environ.get("KERNEL_TRACE", "0")))
    res = run_bass_kernel_spmd(nc, in_maps, core_ids=list(range(NCORES)),
                               trace=trace)
    _cached["last_result"] = res
    outs = [res.results[c]["out"] for c in range(NCORES)]
    return np.concatenate(outs, axis=0)[:N].astype(np.float32)
